# revision 23
# baseline (speedup 1.0000x reference)
"""Trainium2 Bass kernel for 2-layer GATv2 (nn_GATModel): 8-core SPMD.

Sharding: 50000 dst nodes are permuted into 8 cores x NB blocks x 128 slots
(degree-snake balancing so every block has a near-equal incident-edge count).
Each core processes the edges incident to its blocks.

All hot-path data is bf16 (PE matmuls run 4x faster than fp32; DVE gets
2-4x modes); elementwise work is batched per block ([P, T*256] single
instructions); the edge mask is folded into the softmax logits as a -30
bias before exp; the alpha broadcast runs on the scalar engine (exp with a
stride-0 broadcast read). The block loop is software-pipelined 4 ways
(gather / z+prelu / logits / scatter+epilogue) with the pure-DMA gather
stage running 4 blocks ahead so the GpSimd SWDGE stream (~1.2us per
128-row indirect gather, the wall at ~2.1ms/core) stays saturated. The
dense-prologue stores are spread across the scalar/gpsimd DMA queues
(gpsimd is idle until the edge phases) to unclog the sync sequencer.
"""

import math
import os
import sys

import numpy as np

sys.path.insert(0, os.path.dirname(os.path.abspath(__file__)))

import concourse.bass as bass  # noqa: E402
import concourse.mybir as mybir  # noqa: E402
import concourse.tile as tile  # noqa: E402
from concourse.bass import IndirectOffsetOnAxis  # noqa: E402
from concourse.bass_utils import run_bass_kernel_spmd  # noqa: E402

# ---- inlined environment workarounds (must be self-contained) ----
import concourse.tile as _ctile
from concourse import mybir as _mybir
from concourse.vector_clock import ScopedClock as _ScopedClock


def _drain_and_barrier_split(self, tick_clock, wait_clock):
    nc = self.nc
    carrier = nc.sync.nop(nofuse=True, hint="tile_exit_waits")
    wait_clock.add_sem_waits(carrier.ins, _ScopedClock({None: tick_clock.global_clock}))
    waits = list(carrier.ins.sync_info.on_wait)
    carrier.ins.sync_info.on_wait = waits[:1]
    for i in range(1, len(waits)):
        extra = nc.sync.nop(nofuse=True, hint="tile_exit_waits")
        if extra.ins.sync_info is None:
            extra.ins.sync_info = _mybir.SyncInfo(on_wait=waits[i : i + 1], on_update=[])
        else:
            extra.ins.sync_info.on_wait = waits[i : i + 1]
    nc.sync.drain()
    nc.all_engine_barrier()
    popped = nc._tile_sem_poison_stack.pop()
    assert popped is self._sem_poison
    nc.clear_and_free_semaphores(list(self.sems.allocated().values()))
    nc.all_engine_barrier()


def _install_ntff_hook():
    import sys as _s, types as _t
    import antenv
    if "antenv.axon_hooks" in _s.modules:
        return
    mod = _t.ModuleType("antenv.axon_hooks")
    _hook = [None]
    mod.set_axon_ntff_profile_hook = lambda h: _hook.__setitem__(0, h)
    mod.get_axon_ntff_profile_hook = lambda: _hook[0]
    _s.modules["antenv.axon_hooks"] = mod
    antenv.axon_hooks = mod
    try:
        from trn_agent_boot.trn_boot import _ntff_profile_via_ctypes
        mod.set_axon_ntff_profile_hook(_ntff_profile_via_ctypes("/opt/axon/libaxon_pjrt.so"))
        from concourse import bass_utils as _bu
        _bu.upload_artifacts = lambda tmpdir: f"file://{tmpdir}"
    except Exception:
        pass


def _install_patches():
    _ctile.TileContext._drain_and_barrier = _drain_and_barrier_split
    import sys as _s
    _s.path.insert(0, "/root/.axon_site")
    try:
        _install_ntff_hook()
    except Exception as e:
        print("ntff hook install failed:", e)


_install_patches()
# ---- end inlined workarounds ----


F32 = mybir.dt.float32
BF16 = mybir.dt.bfloat16
I32 = mybir.dt.int32
P = 128
NC = 8
HEADS = 4
HID = 64
D1 = HEADS * HID  # 256
IN_DIM = 160
NEG = 0.2
AX = mybir.AxisListType
AF = mybir.ActivationFunctionType
OP = mybir.AluOpType


def _split_multi_waits(nc):
    """This walrus build allows only ONE sem-wait per instruction: hoist
    extra waits onto nofuse NoOps inserted just before, on the same engine."""
    n = 0
    for fn in nc.m.functions:
        for blk in fn.blocks:
            todo = []
            for idx, inst in enumerate(blk.instructions):
                si = inst.sync_info
                if si is not None and len(si.on_wait) > 1:
                    todo.append((idx, inst))
            for idx, inst in reversed(todo):
                waits = list(inst.sync_info.on_wait)
                inst.sync_info.on_wait = waits[-1:]
                for w in waits[:-1]:
                    nop = mybir.InstNoOp(name=f"I-wsplit-{n}", ins=[], outs=[])
                    n += 1
                    nop.engine = inst.engine
                    nop.bass_nofuse = True
                    nop.sync_info = mybir.SyncInfo(on_wait=[w], on_update=[])
                    nc.register_instruction(nop)
                    blk.instructions.insert(idx, nop)
    return n


def _build(NB, TPB, NF):
    """One SPMD program. NB blocks/core, TPB edge tiles/block, NF full-node
    128-tiles for the xl1 table."""
    nc = bass.Bass()
    dp = nc.declare_dram_parameter
    NLOC = NB * P
    T4 = TPB * HEADS

    xTf = dp("xTf", [IN_DIM, NF * P], BF16, isOutput=False)
    xTl = dp("xTl", [IN_DIM, NLOC], BF16, isOutput=False)
    Wl1 = dp("Wl1", [IN_DIM, D1], BF16, isOutput=False)
    Wr1 = dp("Wr1", [IN_DIM, D1], BF16, isOutput=False)
    Wl2 = dp("Wl2", [D1, HID], BF16, isOutput=False)
    Wr2 = dp("Wr2", [D1, HID], BF16, isOutput=False)
    Ws = dp("Ws", [IN_DIM, HID], BF16, isOutput=False)
    src1 = dp("src1", [NB, P, TPB], I32, isOutput=False)
    src2 = dp("src2", [NB, P, TPB], I32, isOutput=False)
    dstr = dp("dstr", [NB, P, TPB], BF16, isOutput=False)
    emb = dp("emb", [NB, P, TPB], BF16, isOutput=False)  # 0 real / -30 pad
    ident = dp("ident", [P, P], BF16, isOutput=False)
    identf = dp("identf", [P, P], F32, isOutput=False)
    iotaF = dp("iotaF", [P, P], BF16, isOutput=False)
    iotaP = dp("iotaP", [P, P], BF16, isOutput=False)
    dstrT = dp("dstrT", [NB, P, TPB * P], BF16, isOutput=False)
    att1b = dp("att1b", [P, D1], BF16, isOutput=False)
    b1rb = dp("b1rb", [P, D1], BF16, isOutput=False)
    s1b = dp("s1b", [P, D1], BF16, isOutput=False)
    c1b = dp("c1b", [P, D1], BF16, isOutput=False)
    att2b = dp("att2b", [P, HID], BF16, isOutput=False)
    b2rb = dp("b2rb", [P, HID], BF16, isOutput=False)
    s2b = dp("s2b", [P, HID], F32, isOutput=False)
    c2b = dp("c2b", [P, HID], F32, isOutput=False)
    bsb = dp("bsb", [P, HID], F32, isOutput=False)
    wob = dp("wob", [P, HID], F32, isOutput=False)
    bob = dp("bob", [P, 1], F32, isOutput=False)
    y = dp("y", [NLOC, 1], F32, isOutput=True)

    xl1d = nc.dram_tensor("xl1d", [NF * P, D1], BF16)
    xr1d = nc.dram_tensor("xr1d", [NLOC, D1], BF16)
    skipd = nc.dram_tensor("skipd", [NLOC, HID], F32)
    xr2d = nc.dram_tensor("xr2d", [NLOC, HID], BF16)
    xl2l = nc.dram_tensor("xl2l", [NLOC, HID], BF16)
    xl2ag = nc.dram_tensor("xl2ag", [NC * NLOC, HID], BF16, addr_space="Shared")

    with tile.TileContext(nc) as tc:
        with (
            tc.tile_pool(name="consts", bufs=1) as cp,
            tc.tile_pool(name="densein", bufs=3) as dip,
            tc.tile_pool(name="denseout", bufs=3) as dop,
            tc.tile_pool(name="gath", bufs=4) as gp,
            tc.tile_pool(name="mid", bufs=2) as mp,
            tc.tile_pool(name="blk", bufs=3) as bp,
            tc.tile_pool(name="small", bufs=2) as sp,
            tc.tile_pool(name="ps_z", bufs=4, space="PSUM") as ps_z,
            tc.tile_pool(name="ps_o", bufs=2, space="PSUM") as ps_o,
            tc.tile_pool(name="ps_t", bufs=2, space="PSUM") as ps_t,
        ):
            # resident constants
            idt = cp.tile([P, P], BF16)
            nc.sync.dma_start(out=idt[:], in_=ident[:])
            idtf = cp.tile([P, P], F32)
            nc.sync.dma_start(out=idtf[:], in_=identf[:])
            iot = cp.tile([P, P], BF16)
            nc.sync.dma_start(out=iot[:], in_=iotaF[:])
            iop = cp.tile([P, P], BF16)
            nc.sync.dma_start(out=iop[:], in_=iotaP[:])
            wl1a = cp.tile([P, D1], BF16)
            nc.sync.dma_start(out=wl1a[:], in_=Wl1[0:P, :])
            wl1b = cp.tile([32, D1], BF16)
            nc.sync.dma_start(out=wl1b[:], in_=Wl1[P:IN_DIM, :])
            wr1a = cp.tile([P, D1], BF16)
            nc.sync.dma_start(out=wr1a[:], in_=Wr1[0:P, :])
            wr1b = cp.tile([32, D1], BF16)
            nc.sync.dma_start(out=wr1b[:], in_=Wr1[P:IN_DIM, :])
            wsa = cp.tile([P, HID], BF16)
            nc.sync.dma_start(out=wsa[:], in_=Ws[0:P, :])
            wsb = cp.tile([32, HID], BF16)
            nc.sync.dma_start(out=wsb[:], in_=Ws[P:IN_DIM, :])
            wl2a = cp.tile([P, HID], BF16)
            nc.sync.dma_start(out=wl2a[:], in_=Wl2[0:P, :])
            wl2b = cp.tile([P, HID], BF16)
            nc.sync.dma_start(out=wl2b[:], in_=Wl2[P:D1, :])
            wr2a = cp.tile([P, HID], BF16)
            nc.sync.dma_start(out=wr2a[:], in_=Wr2[0:P, :])
            wr2b = cp.tile([P, HID], BF16)
            nc.sync.dma_start(out=wr2b[:], in_=Wr2[P:D1, :])
            at1 = cp.tile([P, D1], BF16)
            nc.sync.dma_start(out=at1[:], in_=att1b[:])
            b1r = cp.tile([P, D1], BF16)
            nc.sync.dma_start(out=b1r[:], in_=b1rb[:])
            s1t = cp.tile([P, D1], BF16)
            nc.sync.dma_start(out=s1t[:], in_=s1b[:])
            c1t = cp.tile([P, D1], BF16)
            nc.sync.dma_start(out=c1t[:], in_=c1b[:])
            at2 = cp.tile([P, HID], BF16)
            nc.sync.dma_start(out=at2[:], in_=att2b[:])
            b2r = cp.tile([P, HID], BF16)
            nc.sync.dma_start(out=b2r[:], in_=b2rb[:])
            s2t = cp.tile([P, HID], F32)
            nc.sync.dma_start(out=s2t[:], in_=s2b[:])
            c2t = cp.tile([P, HID], F32)
            nc.sync.dma_start(out=c2t[:], in_=c2b[:])
            bst = cp.tile([P, HID], F32)
            nc.sync.dma_start(out=bst[:], in_=bsb[:])
            wot = cp.tile([P, HID], F32)
            nc.sync.dma_start(out=wot[:], in_=wob[:])
            bot = cp.tile([P, 1], F32)
            nc.sync.dma_start(out=bot[:], in_=bob[:])

            # ---- phase A: xl1 table for ALL nodes, 4 tiles per DMA ----
            GA = 8
            assert NF % GA != -1
            mfull = (NF // GA) * GA
            groups = [(g, GA) for g in range(0, mfull, GA)]
            if mfull < NF:
                groups.append((mfull, NF - mfull))
            for g, gn in groups:
                xa = dip.tile([P, GA * P], BF16, tag="xa")
                nc.sync.dma_start(
                    out=xa[:, 0 : gn * P], in_=xTf[0:P, g * P : (g + gn) * P]
                )
                xb = dip.tile([32, GA * P], BF16, tag="xb")
                nc.sync.dma_start(
                    out=xb[:, 0 : gn * P], in_=xTf[P:IN_DIM, g * P : (g + gn) * P]
                )
                ot = dop.tile([P, GA * D1], BF16, tag="oa")
                for j in range(gn):
                    pz = ps_z.tile([P, D1], F32, tag="pz")
                    nc.tensor.matmul(
                        out=pz[:], lhsT=xa[:, j * P : (j + 1) * P], rhs=wl1a[:],
                        start=True, stop=False,
                    )
                    nc.tensor.matmul(
                        out=pz[:], lhsT=xb[:, j * P : (j + 1) * P], rhs=wl1b[:],
                        start=False, stop=True,
                    )
                    if j % 2 == 0:
                        nc.scalar.copy(out=ot[:, j * D1 : (j + 1) * D1], in_=pz[:])
                    else:
                        nc.vector.tensor_scalar(
                            out=ot[:, j * D1 : (j + 1) * D1], in0=pz[:],
                            scalar1=1.0, scalar2=None, op0=OP.mult,
                        )
                eng = nc.scalar if (g // GA) % 2 == 0 else nc.gpsimd
                eng.dma_start(
                    out=xl1d[g * P : (g + gn) * P, :].rearrange(
                        "(t q) d -> q t d", q=P
                    ),
                    in_=ot[:, 0 : gn * D1].rearrange("q (t d) -> q t d", d=D1),
                )

            # ---- phase B: local xr1 (with bl1+br1 folded) and skip ----
            for m in range(NB):
                xa = dip.tile([P, P], BF16, tag="xa")
                nc.sync.dma_start(out=xa[:], in_=xTl[0:P, m * P : (m + 1) * P])
                xb = dip.tile([32, P], BF16, tag="xb")
                nc.sync.dma_start(out=xb[:], in_=xTl[P:IN_DIM, m * P : (m + 1) * P])
                pz = ps_z.tile([P, D1], F32, tag="pz")
                nc.tensor.matmul(out=pz[:], lhsT=xa[:], rhs=wr1a[:], start=True, stop=False)
                nc.tensor.matmul(out=pz[:], lhsT=xb[:], rhs=wr1b[:], start=False, stop=True)
                ot = dop.tile([P, D1], BF16, tag="oa")
                nc.vector.tensor_add(out=ot[:], in0=pz[:], in1=b1r[:])
                nc.scalar.dma_start(out=xr1d[m * P : (m + 1) * P, :], in_=ot[:])
                pk = ps_t.tile([P, P], F32, tag="poh")
                nc.tensor.matmul(out=pk[:, 0:HID], lhsT=xa[:], rhs=wsa[:], start=True, stop=False)
                nc.tensor.matmul(out=pk[:, 0:HID], lhsT=xb[:], rhs=wsb[:], start=False, stop=True)
                st = dop.tile([P, HID], F32, tag="sk")
                nc.vector.tensor_add(out=st[:], in0=pk[:, 0:HID], in1=bst[:])
                nc.gpsimd.dma_start(out=skipd[m * P : (m + 1) * P, :], in_=st[:])

            # ---- resident per-block metadata (one DMA per table) ----
            s1r = cp.tile([P, NB * TPB], I32)
            nc.sync.dma_start(out=s1r[:].rearrange("p (b t) -> p b t", t=TPB),
                              in_=src1[:].rearrange("b p t -> p b t"))
            s2r = cp.tile([P, NB * TPB], I32)
            nc.sync.dma_start(out=s2r[:].rearrange("p (b t) -> p b t", t=TPB),
                              in_=src2[:].rearrange("b p t -> p b t"))
            dsrr = cp.tile([P, NB * TPB], BF16)
            nc.sync.dma_start(out=dsrr[:].rearrange("p (b t) -> p b t", t=TPB),
                              in_=dstr[:].rearrange("b p t -> p b t"))
            mbr = cp.tile([P, NB * TPB], BF16)
            nc.sync.dma_start(out=mbr[:].rearrange("p (b t) -> p b t", t=TPB),
                              in_=emb[:].rearrange("b p t -> p b t"))

            # ---- phase C: conv1 edge stage, software-pipelined ----
            def c_front(b):
                st = {}
                xrb = bp.tile([P, D1], BF16, tag="xrb")
                nc.sync.dma_start(out=xrb[:], in_=xr1d[b * P : (b + 1) * P, :])
                st["xrb"] = xrb
                dsT = bp.tile([P, TPB * P], BF16, tag="dsT")
                nc.sync.dma_start(out=dsT[:], in_=dstrT[b])
                TA = (TPB + 1) // 2
                xltA = gp.tile([P, TA * D1], BF16, tag="xltA")
                xltB = gp.tile([P, (TPB - TA) * D1], BF16, tag="xltB")
                for k in range(TA):
                    for half, t in ((0, k), (1, TA + k)):
                        if t >= TPB:
                            continue
                        buf = xltA if half == 0 else xltB
                        off = t if half == 0 else t - TA
                        nc.gpsimd.indirect_dma_start(
                            out=buf[:, off * D1 : (off + 1) * D1],
                            out_offset=None,
                            in_=xl1d[:],
                            in_offset=IndirectOffsetOnAxis(
                                ap=s1r[:, b * TPB + t : b * TPB + t + 1], axis=0
                            ),
                        )
                st["xltA"], st["xltB"], st["TA"] = xltA, xltB, TA
                st["dsT"] = dsT
                return st

            def c_fz(b, st):
                xrb, TA, dsT = st["xrb"], st["TA"], st["dsT"]
                ohT = mp.tile([P, TPB * P], BF16, tag="ohT")
                nc.vector.tensor_tensor(
                    out=ohT[:].rearrange("p (t q) -> p t q", q=P),
                    in0=iop[:].rearrange("p (o q) -> p o q", o=1).to_broadcast([P, TPB, P]),
                    in1=dsT[:].rearrange("p (t q) -> p t q", q=P),
                    op=OP.is_equal,
                )
                oh = mp.tile([P, TPB * P], BF16, tag="oh")
                nc.vector.tensor_tensor(
                    out=oh[:].rearrange("p (t q) -> p t q", q=P),
                    in0=dsrr[:, b * TPB : (b + 1) * TPB]
                    .rearrange("p (t o) -> p t o", o=1).to_broadcast([P, TPB, P]),
                    in1=iot[:].rearrange("p (o q) -> p o q", o=1).to_broadcast([P, TPB, P]),
                    op=OP.is_equal,
                )
                st["oh"] = oh
                lr = mp.tile([P, TPB * D1], BF16, tag="lr")
                for t in range(TPB):
                    xsrc = (st["xltA"][:, t * D1 : (t + 1) * D1] if t < TA
                            else st["xltB"][:, (t - TA) * D1 : (t - TA + 1) * D1])
                    pz = ps_z.tile([P, D1], F32, tag="pz")
                    nc.tensor.matmul(
                        out=pz[:], lhsT=idt[:], rhs=xsrc,
                        start=True, stop=False,
                    )
                    nc.tensor.matmul(
                        out=pz[:], lhsT=ohT[:, t * P : (t + 1) * P], rhs=xrb[:],
                        start=False, stop=True,
                    )
                    nc.scalar.activation(
                        out=lr[:, t * D1 : (t + 1) * D1], in_=pz[:], func=AF.Prelu,
                        alpha=NEG,
                    )
                st["lr"] = lr

            def c_back(b, st):
                oh, lr, xrb = st["oh"], st["lr"], st["xrb"]
                tt = mp.tile([P, TPB * D1], BF16, tag="tt")
                nc.vector.tensor_tensor(
                    out=tt[:].rearrange("p (t d) -> p t d", d=D1),
                    in0=lr[:].rearrange("p (t d) -> p t d", d=D1),
                    in1=at1[:].rearrange("p (o d) -> p o d", o=1).to_broadcast([P, TPB, D1]),
                    op=OP.mult,
                )
                lg = sp.tile([P, T4], BF16, tag="lg")
                with nc.allow_low_precision("bf16 logits within 2e-2 tol"):
                    nc.vector.tensor_reduce(
                        out=lg[:].rearrange("p (t h) -> p t h", h=HEADS),
                        in_=tt[:].rearrange("p (t h d) -> p t h d", h=HEADS, d=HID),
                        axis=AX.X, op=OP.add,
                    )
                lg2 = sp.tile([P, T4], BF16, tag="lg2")
                nc.vector.tensor_tensor(
                    out=lg2[:].rearrange("p (t h) -> p t h", h=HEADS),
                    in0=lg[:].rearrange("p (t h) -> p t h", h=HEADS),
                    in1=mbr[:, b * TPB : (b + 1) * TPB]
                    .rearrange("p (t o) -> p t o", o=1).to_broadcast([P, TPB, HEADS]),
                    op=OP.add,
                )
                v = mp.tile([P, TPB * (D1 + HEADS)], BF16, tag="v")
                vv = v[:].rearrange("p (t d) -> p t d", d=D1 + HEADS)
                nc.scalar.activation(
                    out=vv[:, :, D1 : D1 + HEADS],
                    in_=lg2[:].rearrange("p (t h) -> p t h", h=HEADS),
                    func=AF.Exp,
                )
                e4x = mp.tile([P, TPB * D1], BF16, tag="e4x")
                nc.scalar.activation(
                    out=e4x[:].rearrange("p (t h d) -> p t h d", h=HEADS, d=HID),
                    in_=lg2[:].rearrange("p (t h o) -> p t h o", h=HEADS, o=1)
                    .to_broadcast([P, TPB, HEADS, HID]),
                    func=AF.Exp,
                )
                TA = st["TA"]
                nc.vector.tensor_tensor(
                    out=vv[:, 0:TA, 0:D1],
                    in0=st["xltA"][:].rearrange("p (t d) -> p t d", d=D1),
                    in1=e4x[:, 0 : TA * D1].rearrange("p (t d) -> p t d", d=D1),
                    op=OP.mult,
                )
                nc.vector.tensor_tensor(
                    out=vv[:, TA:TPB, 0:D1],
                    in0=st["xltB"][:].rearrange("p (t d) -> p t d", d=D1),
                    in1=e4x[:, TA * D1 :].rearrange("p (t d) -> p t d", d=D1),
                    op=OP.mult,
                )
                st["v"] = v

            def c_tail(b, st):
                oh, v = st["oh"], st["v"]
                pout = ps_o.tile([P, D1 + HEADS], F32, tag="pout")
                for t in range(TPB):
                    nc.tensor.matmul(
                        out=pout[:],
                        lhsT=oh[:, t * P : (t + 1) * P],
                        rhs=v[:, t * (D1 + HEADS) : (t + 1) * (D1 + HEADS)],
                        start=(t == 0),
                        stop=(t == TPB - 1),
                    )
                r4 = sp.tile([P, HEADS], F32, tag="r4")
                nc.vector.reciprocal(out=r4[:], in_=pout[:, D1 : D1 + HEADS])
                hsbf = mp.tile([P, D1], F32, tag="hsb")
                hsb = hsbf
                for h in range(HEADS):
                    nc.vector.tensor_scalar(
                        out=hsb[:, h * HID : (h + 1) * HID],
                        in0=pout[:, h * HID : (h + 1) * HID],
                        scalar1=r4[:, h : h + 1],
                        scalar2=None,
                        op0=OP.mult,
                    )
                nc.vector.tensor_mul(out=hsb[:], in0=hsb[:], in1=s1t[:])
                nc.vector.tensor_add(out=hsb[:], in0=hsb[:], in1=c1t[:])
                nc.vector.tensor_scalar(
                    out=hsb[:], in0=hsb[:], scalar1=0.0, scalar2=None, op0=OP.max
                )
                pt1 = ps_t.tile([P, P], F32, tag="poh")
                nc.tensor.transpose(out=pt1[:], in_=hsbf[:, 0:P], identity=idtf[:])
                hT1 = sp.tile([P, P], BF16, tag="hT1")
                nc.scalar.copy(out=hT1[:], in_=pt1[:])
                pt2 = ps_t.tile([P, P], F32, tag="poh")
                nc.tensor.transpose(out=pt2[:], in_=hsbf[:, P:D1], identity=idtf[:])
                hT2 = sp.tile([P, P], BF16, tag="hT2")
                nc.scalar.copy(out=hT2[:], in_=pt2[:])
                pl2 = ps_t.tile([P, P], F32, tag="poh")
                nc.tensor.matmul(out=pl2[:, 0:HID], lhsT=hT1[:], rhs=wl2a[:], start=True, stop=False)
                nc.tensor.matmul(out=pl2[:, 0:HID], lhsT=hT2[:], rhs=wl2b[:], start=False, stop=True)
                l2t = dop.tile([P, HID], BF16, tag="l2t")
                nc.scalar.copy(out=l2t[:], in_=pl2[:, 0:HID])
                nc.sync.dma_start(out=xl2l[b * P : (b + 1) * P, :], in_=l2t[:])
                pr2 = ps_t.tile([P, P], F32, tag="poh")
                nc.tensor.matmul(out=pr2[:, 0:HID], lhsT=hT1[:], rhs=wr2a[:], start=True, stop=False)
                nc.tensor.matmul(out=pr2[:, 0:HID], lhsT=hT2[:], rhs=wr2b[:], start=False, stop=True)
                r2t = dop.tile([P, HID], BF16, tag="r2t")
                nc.vector.tensor_add(out=r2t[:], in0=pr2[:, 0:HID], in1=b2r[:])
                nc.sync.dma_start(out=xr2d[b * P : (b + 1) * P, :], in_=r2t[:])

            stc = {}
            for j in range(4):
                stc[j] = c_front(j)
            c_fz(0, stc[0])
            for b in range(NB):
                c_back(b, stc[b])
                if b + 1 < NB:
                    c_fz(b + 1, stc[b + 1])
                c_tail(b, stc[b])
                if b + 4 < NB:
                    stc[b + 4] = c_front(b + 4)
                del stc[b]

            # ---- phase E: exchange layer-2 source features ----
            nc.gpsimd.collective_compute(
                "AllGather",
                OP.bypass,
                replica_groups=[list(range(NC))],
                ins=[xl2l[:]],
                outs=[xl2ag[:]],
            )

            # ---- phase F: conv2 edge stage, software-pipelined ----
            def f_front(b):
                st = {}
                xr2b = bp.tile([P, HID], BF16, tag="xr2b")
                nc.sync.dma_start(out=xr2b[:], in_=xr2d[b * P : (b + 1) * P, :])
                st["xr2b"] = xr2b
                skt = bp.tile([P, HID], F32, tag="skt")
                nc.sync.dma_start(out=skt[:], in_=skipd[b * P : (b + 1) * P, :])
                st["skt"] = skt
                dsT = bp.tile([P, TPB * P], BF16, tag="dsT")
                nc.sync.dma_start(out=dsT[:], in_=dstrT[b])
                TA = (TPB + 1) // 2
                xltA = gp.tile([P, TA * HID], BF16, tag="xl2tA")
                xltB = gp.tile([P, (TPB - TA) * HID], BF16, tag="xl2tB")
                for k in range(TA):
                    for half, t in ((0, k), (1, TA + k)):
                        if t >= TPB:
                            continue
                        buf = xltA if half == 0 else xltB
                        off = t if half == 0 else t - TA
                        nc.gpsimd.indirect_dma_start(
                            out=buf[:, off * HID : (off + 1) * HID],
                            out_offset=None,
                            in_=xl2ag[:],
                            in_offset=IndirectOffsetOnAxis(
                                ap=s2r[:, b * TPB + t : b * TPB + t + 1], axis=0
                            ),
                        )
                st["xltA"], st["xltB"], st["TA"] = xltA, xltB, TA
                st["dsT"] = dsT
                return st

            def f_fz(b, st):
                xr2b, TA, dsT = st["xr2b"], st["TA"], st["dsT"]
                ohT = mp.tile([P, TPB * P], BF16, tag="ohT")
                nc.vector.tensor_tensor(
                    out=ohT[:].rearrange("p (t q) -> p t q", q=P),
                    in0=iop[:].rearrange("p (o q) -> p o q", o=1).to_broadcast([P, TPB, P]),
                    in1=dsT[:].rearrange("p (t q) -> p t q", q=P),
                    op=OP.is_equal,
                )
                oh = mp.tile([P, TPB * P], BF16, tag="oh")
                nc.vector.tensor_tensor(
                    out=oh[:].rearrange("p (t q) -> p t q", q=P),
                    in0=dsrr[:, b * TPB : (b + 1) * TPB]
                    .rearrange("p (t o) -> p t o", o=1).to_broadcast([P, TPB, P]),
                    in1=iot[:].rearrange("p (o q) -> p o q", o=1).to_broadcast([P, TPB, P]),
                    op=OP.is_equal,
                )
                st["oh"] = oh
                lr = mp.tile([P, TPB * HID], BF16, tag="lr2")
                for t in range(TPB):
                    xsrc = (st["xltA"][:, t * HID : (t + 1) * HID] if t < TA
                            else st["xltB"][:, (t - TA) * HID : (t - TA + 1) * HID])
                    pzw = ps_z.tile([P, D1], F32, tag="pz")
                    pz = pzw[:, 0:HID]
                    nc.tensor.matmul(
                        out=pz, lhsT=idt[:], rhs=xsrc,
                        start=True, stop=False,
                    )
                    nc.tensor.matmul(
                        out=pz, lhsT=ohT[:, t * P : (t + 1) * P], rhs=xr2b[:],
                        start=False, stop=True,
                    )
                    nc.scalar.activation(
                        out=lr[:, t * HID : (t + 1) * HID], in_=pz, func=AF.Prelu,
                        alpha=NEG,
                    )
                st["lr"] = lr

            def f_back(b, st):
                oh, lr = st["oh"], st["lr"]
                tt = mp.tile([P, TPB * HID], BF16, tag="tt2")
                nc.vector.tensor_tensor(
                    out=tt[:].rearrange("p (t d) -> p t d", d=HID),
                    in0=lr[:].rearrange("p (t d) -> p t d", d=HID),
                    in1=at2[:].rearrange("p (o d) -> p o d", o=1).to_broadcast([P, TPB, HID]),
                    op=OP.mult,
                )
                lg = sp.tile([P, TPB], BF16, tag="lgf")
                with nc.allow_low_precision("bf16 logits within 2e-2 tol"):
                    nc.vector.tensor_reduce(
                        out=lg[:],
                        in_=tt[:].rearrange("p (t d) -> p t d", d=HID),
                        axis=AX.X, op=OP.add,
                    )
                lg2 = sp.tile([P, TPB], BF16, tag="lg2f")
                nc.vector.tensor_add(out=lg2[:], in0=lg[:], in1=mbr[:, b * TPB : (b + 1) * TPB])
                v = mp.tile([P, TPB * (HID + 1)], BF16, tag="v2")
                vv = v[:].rearrange("p (t d) -> p t d", d=HID + 1)
                nc.scalar.activation(
                    out=vv[:, :, HID : HID + 1],
                    in_=lg2[:].rearrange("p (t o) -> p t o", o=1),
                    func=AF.Exp,
                )
                e1x = mp.tile([P, TPB * HID], BF16, tag="e1x")
                nc.scalar.activation(
                    out=e1x[:].rearrange("p (t d) -> p t d", d=HID),
                    in_=lg2[:].rearrange("p (t o) -> p t o", o=1).to_broadcast([P, TPB, HID]),
                    func=AF.Exp,
                )
                TA = st["TA"]
                nc.vector.tensor_tensor(
                    out=vv[:, 0:TA, 0:HID],
                    in0=st["xltA"][:].rearrange("p (t d) -> p t d", d=HID),
                    in1=e1x[:, 0 : TA * HID].rearrange("p (t d) -> p t d", d=HID),
                    op=OP.mult,
                )
                nc.vector.tensor_tensor(
                    out=vv[:, TA:TPB, 0:HID],
                    in0=st["xltB"][:].rearrange("p (t d) -> p t d", d=HID),
                    in1=e1x[:, TA * HID :].rearrange("p (t d) -> p t d", d=HID),
                    op=OP.mult,
                )
                st["v"] = v

            def f_tail(b, st):
                oh, v = st["oh"], st["v"]
                pout = ps_o.tile([P, D1 + HEADS], F32, tag="pout")
                for t in range(TPB):
                    nc.tensor.matmul(
                        out=pout[:, 0 : HID + 1],
                        lhsT=oh[:, t * P : (t + 1) * P],
                        rhs=v[:, t * (HID + 1) : (t + 1) * (HID + 1)],
                        start=(t == 0),
                        stop=(t == TPB - 1),
                    )
                r1 = sp.tile([P, 1], F32, tag="r1")
                nc.vector.reciprocal(out=r1[:], in_=pout[:, HID : HID + 1])
                h2 = sp.tile([P, HID], F32, tag="h2")
                nc.vector.tensor_scalar(
                    out=h2[:], in0=pout[:, 0:HID], scalar1=r1[:, 0:1], scalar2=None, op0=OP.mult
                )
                nc.vector.tensor_mul(out=h2[:], in0=h2[:], in1=s2t[:])
                nc.vector.tensor_add(out=h2[:], in0=h2[:], in1=c2t[:])
                nc.vector.tensor_scalar(
                    out=h2[:], in0=h2[:], scalar1=0.0, scalar2=None, op0=OP.max
                )
                nc.vector.tensor_add(out=h2[:], in0=h2[:], in1=st["skt"][:])
                nc.vector.tensor_mul(out=h2[:], in0=h2[:], in1=wot[:])
                yp = sp.tile([P, 1], F32, tag="yp")
                nc.vector.reduce_sum(out=yp[:], in_=h2[:], axis=AX.X)
                yb = sp.tile([P, 1], F32, tag="yb")
                nc.vector.tensor_scalar(
                    out=yb[:], in0=yp[:], scalar1=bot[:, 0:1], scalar2=None, op0=OP.add
                )
                nc.sync.dma_start(out=y[b * P : (b + 1) * P, :], in_=yb[:])

            stf = {}
            for j in range(4):
                stf[j] = f_front(j)
            f_fz(0, stf[0])
            for b in range(NB):
                f_back(b, stf[b])
                if b + 1 < NB:
                    f_fz(b + 1, stf[b + 1])
                f_tail(b, stf[b])
                if b + 4 < NB:
                    stf[b + 4] = f_front(b + 4)
                del stf[b]
    _split_multi_waits(nc)
    return nc


def _host_prep(x, edge_index, Wl1, bl1, Wr1, br1, att1, bias1, g1, b1, m1, v1,
               Wl2, bl2, Wr2, br2, att2, bias2, g2, b2, m2, v2, Ws, bs, Wo, bo,
               NB):
    """Numpy-side graph partitioning + constant folding. Returns (in_maps,
    meta) where meta carries the node permutation for un-sharding."""
    import ml_dtypes
    BF = ml_dtypes.bfloat16
    N = x.shape[0]
    src = np.concatenate([edge_index[0], np.arange(N, dtype=np.int64)]).astype(np.int64)
    dst = np.concatenate([edge_index[1], np.arange(N, dtype=np.int64)]).astype(np.int64)

    NBINS = NC * NB
    deg = np.bincount(dst, minlength=N)
    order = np.argsort(-deg, kind="stable")
    i = np.arange(N)
    r = i // NBINS
    pos = i % NBINS
    bsel = np.where(r % 2 == 0, pos, NBINS - 1 - pos)
    binof = np.empty(N, np.int64)
    slotof = np.empty(N, np.int64)
    binof[order] = bsel
    slotof[order] = r
    assert slotof.max() < P

    ebin = binof[dst]
    eorder = np.argsort(ebin, kind="stable")
    counts = np.bincount(ebin, minlength=NBINS)
    TPB = int(math.ceil(counts.max() / P))
    offs = np.zeros(NBINS + 1, np.int64)
    np.cumsum(counts, out=offs[1:])
    pwc = np.arange(len(src)) - offs[ebin[eorder]]

    src_arr = np.zeros((NBINS, P, TPB), np.int32)
    src2_arr = np.zeros((NBINS, P, TPB), np.int32)
    dst_arr = np.zeros((NBINS, P, TPB), np.float32)
    mb_arr = np.full((NBINS, P, TPB), -30.0, np.float32)
    pp = (pwc % P).astype(np.int64)
    tt = (pwc // P).astype(np.int64)
    eb = ebin[eorder]
    es = src[eorder]
    ed = dst[eorder]
    agof = (binof // NB) * (NB * P) + (binof % NB) * P + slotof  # node -> allgather row
    src_arr[eb, pp, tt] = es.astype(np.int32)
    src2_arr[eb, pp, tt] = agof[es].astype(np.int32)
    dst_arr[eb, pp, tt] = slotof[ed].astype(np.float32)
    mb_arr[eb, pp, tt] = 0.0

    NF = (N + P - 1) // P
    xTf = np.zeros((IN_DIM, NF * P), np.float32)
    xTf[:, :N] = x.T

    # local x (block, slot) layout per core
    node_of = np.full((NBINS, P), -1, np.int64)
    node_of[binof, slotof] = np.arange(N)

    s1 = (g1 / np.sqrt(v1 + 1e-5)).astype(np.float32)
    c1 = ((bias1 + bl1 - m1) * s1 + b1).astype(np.float32)
    s2 = (g2 / np.sqrt(v2 + 1e-5)).astype(np.float32)
    c2 = ((bias2 + bl2 - m2) * s2 + b2).astype(np.float32)
    rep = lambda v: np.tile(np.asarray(v, np.float32).reshape(1, -1), (P, 1))

    common = dict(
        xTf=xTf.astype(BF), Wl1=np.asarray(Wl1, np.float32).astype(BF),
        Wr1=np.asarray(Wr1, np.float32).astype(BF),
        Wl2=np.asarray(Wl2, np.float32).astype(BF),
        Wr2=np.asarray(Wr2, np.float32).astype(BF),
        Ws=np.asarray(Ws, np.float32).astype(BF),
        ident=np.eye(P, dtype=np.float32).astype(BF),
        identf=np.eye(P, dtype=np.float32),
        iotaF=np.tile(np.arange(P, dtype=np.float32).reshape(1, -1), (P, 1)).astype(BF),
        iotaP=np.tile(np.arange(P, dtype=np.float32).reshape(-1, 1), (1, P)).astype(BF),
        att1b=rep(np.asarray(att1, np.float32).reshape(-1)).astype(BF),
        b1rb=rep(bl1 + br1).astype(BF), s1b=rep(s1).astype(BF), c1b=rep(c1).astype(BF),
        att2b=rep(np.asarray(att2, np.float32).reshape(-1)).astype(BF),
        b2rb=rep(bl2 + br2).astype(BF), s2b=rep(s2), c2b=rep(c2),
        bsb=rep(bs), wob=rep(np.asarray(Wo, np.float32).reshape(-1)),
        bob=np.full((P, 1), np.float32(np.asarray(bo).reshape(-1)[0])),
    )

    in_maps = []
    for c in range(NC):
        bins = slice(c * NB, (c + 1) * NB)
        nid = node_of[bins].reshape(-1)  # [NB*P]
        xl = np.zeros((NB * P, IN_DIM), np.float32)
        ok = nid >= 0
        xl[ok] = x[nid[ok]]
        m = dict(common)
        dT = dst_arr[bins].transpose(0, 2, 1).reshape(NB, 1, TPB * P)
        m.update(
            xTl=np.ascontiguousarray(xl.T).astype(BF),
            src1=src_arr[bins], src2=src2_arr[bins],
            dstr=dst_arr[bins].astype(BF), emb=mb_arr[bins].astype(BF),
            dstrT=np.ascontiguousarray(np.broadcast_to(dT, (NB, P, TPB * P))).astype(BF),
        )
        in_maps.append(m)

    meta = dict(TPB=TPB, NF=NF, agof=agof, N=N)
    return in_maps, meta


_PROG_CACHE = {}


def kernel(**inputs):
    NB = 50
    inp = {k: np.asarray(v) for k, v in inputs.items()}
    x = inp["x"].astype(np.float32)
    in_maps, meta = _host_prep(
        x, inp["edge_index"], inp["Wl1"], inp["bl1"], inp["Wr1"], inp["br1"],
        inp["att1"], inp["bias1"], inp["g1"], inp["b1"], inp["m1"], inp["v1"],
        inp["Wl2"], inp["bl2"], inp["Wr2"], inp["br2"], inp["att2"], inp["bias2"],
        inp["g2"], inp["b2"], inp["m2"], inp["v2"], inp["Ws"], inp["bs"],
        inp["Wo"], inp["bo"], NB,
    )
    key = (NB, meta["TPB"], meta["NF"])
    if key not in _PROG_CACHE:
        _PROG_CACHE[key] = _build(*key)
    nc = _PROG_CACHE[key]
    res = run_bass_kernel_spmd(nc, in_maps, list(range(NC)))
    ylin = np.concatenate([res.results[c]["y"].reshape(-1) for c in range(NC)])
    return ylin[meta["agof"]].astype(np.float32)
